# revision 13
# baseline (speedup 1.0000x reference)
"""Trainium2 Bass kernel for nn_LilletLayer (gnn_message_passing).

Math (per molecule b, per head h):
  xc = W_map @ x   (K=6 coarse particles, 3d coords)
  delta over K*K (k1,k2) pairs -> ExpNormalSmearing -> basis (36, 50, 3)
  att[a,c,n] = sum_x basis[a,n,x]*basis[c,n,x]
  out = silu(att @ W1 + b1) @ W2 + b2

Exact algebraic folds (validated vs the reference in fp32):
 1. basis[a,n,x] = deltam[x,a]*g[a,n] is separable, so
      att[a,c,n] = D2[a,c]*g[a,n]*g[c,n],  D2 = deltam^T deltam.
 2. The 6 diagonal (k,k) pairs have delta == 0 exactly; mirror pairs have
    basis = -basis. The (36x36) pair-pair contraction collapses onto the
    120 upper-triangular pair-pairs of the 15 canonical (k1<k2) pairs:
    W1 is folded host-side with the mirror signs and the symmetry
    doubling. Device contraction: 120*50 = 6000 rows per head.
 3. The cutoff/denominator factor m3[a] = (cos(pi*d_a/5)+1)/(d_a+1e-6)^2
    is folded into the pairwise gram: d2fm[a,c] = D2[a,c]*m3[a]*m3[c],
    so att rows are g[a,n]*g[c,n]*d2fm[a,c] (W1 carries the 1/4).

Sharding: one NeuronCore per head (H=8). Each core computes its head's
6000-row bf16 att block in [b, f] layout (DVE), XBAR-DMA-transposes it
to [f, b] chunks, matmuls against resident bf16 folded-W1 chunks
accumulating h1^T[j,b] fp32 in PSUM, adds b1/8, PE-transposes to
[b, j], and ReduceScatters over the batch dim: core r receives the
summed h1 rows for molecules 16r..16r+15, finishes Silu + W2 + b2 on
those 16 rows, and outputs a (16,1) slice. The host concatenates the 8
slices (pure gather).
"""

import math

import numpy as np

import concourse.bacc as bacc
import concourse.bass as bass
import concourse.mybir as mybir
import concourse.tile as tile
from concourse.bass_utils import run_bass_kernel_spmd
from concourse.masks import make_identity

B, N, H, K, R = 128, 512, 8, 6, 50
CUT = 5.0
P15 = K * (K - 1) // 2        # 15 canonical (k1<k2) pairs
FTOT = P15 * (P15 + 1) // 2 * R  # 6000 contraction rows per head
NCH = (FTOT + 127) // 128     # 47 chunks
FPAD = NCH * 128              # 6016 padded rows
SPANS = [(P15 - a) * R for a in range(P15)]
OFFS = np.concatenate([[0], np.cumsum(SPANS)]).astype(int)
GRP = 8                       # transpose/matmul group size (chunks)
NWARM = 30                    # PE clock-ramp warmup transposes
HID = 128
BLOC = B // 8                 # 16 molecules per core after ReduceScatter
F32 = mybir.dt.float32
BF16 = mybir.dt.bfloat16
AF = mybir.ActivationFunctionType
ALU = mybir.AluOpType


def _bcast(ap, axis, count):
    """Insert a stride-0 (broadcast) free dim at free-axis position `axis`."""
    dims = [list(d) for d in ap.ap]
    dims.insert(axis + 1, [0, count])  # +1: dims[0] is the partition dim
    return bass.AP(tensor=ap.tensor, offset=ap.offset, ap=dims)


def _with_dims(ap, dims):
    """Replace the free dims of `ap` with explicit [step, count] pairs."""
    return bass.AP(
        tensor=ap.tensor, offset=ap.offset, ap=[list(ap.ap[0])] + [list(d) for d in dims]
    )


def _mkap(ap, dims):
    """Build an AP over `ap`'s tensor with fully explicit [step, count] dims."""
    return bass.AP(tensor=ap.tensor, offset=ap.offset, ap=[list(d) for d in dims])


def build_program(n_cores=8):
    nc = bacc.Bacc(
        "TRN2",
        target_bir_lowering=False,
        debug=False,
        enable_asserts=False,
        num_devices=n_cores,
    )

    xcin = nc.dram_tensor("xcin", [B, 3, K], F32, kind="ExternalInput").ap()
    w1s = nc.dram_tensor("w1s", [FPAD, HID], BF16, kind="ExternalInput").ap()
    mrep = nc.dram_tensor("mrep", [B, R], F32, kind="ExternalInput").ap()
    nbs = nc.dram_tensor("nbs", [B, 1], F32, kind="ExternalInput").ap()
    b18d = nc.dram_tensor("b18", [HID, 1], F32, kind="ExternalInput").ap()
    w2rd = nc.dram_tensor("w2r", [BLOC, HID], F32, kind="ExternalInput").ap()
    b2rd = nc.dram_tensor("b2r", [BLOC, 1], F32, kind="ExternalInput").ap()
    outd = nc.dram_tensor("out", [BLOC, 1], F32, kind="ExternalOutput").ap()
    ar_in = nc.dram_tensor("ar_in", [B, HID], BF16, kind="Internal").ap()
    ar_out = nc.dram_tensor("ar_out", [BLOC, HID], BF16, kind="Internal").ap()

    with tile.TileContext(nc) as tc:
        with (
            tc.tile_pool(name="singles", bufs=1) as singles,
            tc.tile_pool(name="g2p", bufs=2) as g2p,
            tc.tile_pool(name="attTp", bufs=2) as attT_pool,
            tc.tile_pool(name="ps_acc", bufs=1, space="PSUM") as ps_acc_pool,
            tc.tile_pool(name="ps_t", bufs=2, space="PSUM") as ps_t_pool,
            tc.tile_pool(name="ps_warm", bufs=1, space="PSUM") as ps_warm_pool,
        ):
            # ---------------- input DMAs (spread across issue queues) --------
            xc_sb = singles.tile([128, 3, K], F32)
            nc.sync.dma_start(out=xc_sb, in_=xcin)
            mrep_sb = singles.tile([128, R], F32)
            nc.sync.dma_start(out=mrep_sb, in_=mrep)
            nbs_sb = singles.tile([128, 1], F32)
            nc.sync.dma_start(out=nbs_sb, in_=nbs)
            b18_sb = singles.tile([128, 1], F32)
            nc.sync.dma_start(out=b18_sb, in_=b18d)
            w2r_sb = singles.tile([BLOC, HID], F32)
            nc.sync.dma_start(out=w2r_sb, in_=w2rd)
            b2r_sb = singles.tile([BLOC, 1], F32)
            nc.sync.dma_start(out=b2r_sb, in_=b2rd)
            # all folded-W1 rows resident in SBUF, one big DMA (16 engines)
            w1all = singles.tile([128, NCH, HID], BF16)
            nc.scalar.dma_start(
                out=w1all,
                in_=_mkap(w1s, [[HID, 128], [HID * 128, NCH], [1, HID]]),
            )

            # ---------------- constants + ACT table warming ------------------
            ident = singles.tile([128, 128], BF16)
            make_identity(nc, ident)
            c_halfpi = singles.tile([128, 1], F32)
            nc.vector.memset(c_halfpi, math.pi / 2)
            warm = singles.tile([128, 1], F32)
            nc.scalar.activation(warm, c_halfpi[:, 0:1], AF.Sqrt)
            nc.scalar.activation(warm, c_halfpi[:, 0:1], AF.Sin)
            nc.scalar.activation(warm, c_halfpi[:, 0:1], AF.Exp)

            # ------------- delta over the 15 canonical (k1<k2) pairs ---------
            delta_sb = singles.tile([128, 3, P15], F32)
            off = 0
            for q1 in range(K - 1):
                cnt = K - 1 - q1
                nc.vector.tensor_sub(
                    delta_sb[:, :, off:off + cnt],
                    _bcast(xc_sb[:, :, q1], 1, cnt),
                    xc_sb[:, :, q1 + 1:],
                )
                off += cnt

            # d2[b, a] = sum_d delta^2 ; dnorm = sqrt(d2)
            d2sq_sb = singles.tile([128, P15, 3], F32)
            nc.vector.tensor_mul(
                d2sq_sb,
                _with_dims(delta_sb[:], [[1, P15], [P15, 3]]),
                _with_dims(delta_sb[:], [[1, P15], [P15, 3]]),
            )
            d2_sb = singles.tile([128, P15], F32)
            nc.vector.tensor_reduce(
                d2_sb, d2sq_sb, axis=mybir.AxisListType.X, op=ALU.add
            )
            dnorm_sb = singles.tile([128, P15], F32)
            nc.scalar.activation(dnorm_sb, d2_sb, AF.Sqrt)

            # -------- pairwise gram (DVE): d2f = delta.delta -----------------
            # pure-delta work first, filling the DVE while ACT runs sqrt
            q0 = singles.tile([128, P15, P15], F32)
            nc.vector.tensor_mul(
                q0,
                _with_dims(delta_sb[:, 0], [[1, P15], [0, P15]]),
                _with_dims(delta_sb[:, 0], [[0, P15], [1, P15]]),
            )
            q1t = singles.tile([128, P15, P15], F32)
            nc.vector.tensor_mul(
                q1t,
                _with_dims(delta_sb[:, 1], [[1, P15], [0, P15]]),
                _with_dims(delta_sb[:, 1], [[0, P15], [1, P15]]),
            )
            q01 = singles.tile([128, P15, P15], F32)
            nc.vector.tensor_add(q01, q0, q1t)
            q2 = singles.tile([128, P15, P15], F32)
            nc.vector.tensor_mul(
                q2,
                _with_dims(delta_sb[:, 2], [[1, P15], [0, P15]]),
                _with_dims(delta_sb[:, 2], [[0, P15], [1, P15]]),
            )
            d2f_sb = singles.tile([128, P15, P15], F32)
            nc.vector.tensor_add(d2f_sb, q01, q2)

            # m3 = (cos(pi*min(d,CUT)/CUT) + 1) / (d+1e-6)^2   (DVE smalls)
            pe_sb = singles.tile([128, P15], F32)
            nc.vector.tensor_single_scalar(pe_sb, dnorm_sb, 1e-6, op=ALU.add)
            dc_sb = singles.tile([128, P15], F32)
            nc.vector.tensor_single_scalar(dc_sb, dnorm_sb, CUT, op=ALU.min)
            e_sb = singles.tile([128, P15], F32)
            nc.scalar.activation(e_sb, dnorm_sb, AF.Exp, scale=-1.0)
            c1_sb = singles.tile([128, P15], F32)
            nc.scalar.activation(
                c1_sb, dc_sb, AF.Sin, scale=-math.pi / CUT, bias=c_halfpi[:, 0:1]
            )
            # t next on the DVE queue: it only waits on e (critical chain)
            t_sb = singles.tile([128, P15, R], F32)
            nc.vector.tensor_sub(
                t_sb, _bcast(e_sb[:], 1, R), _bcast(mrep_sb[:], 0, P15)
            )
            p2_sb = singles.tile([128, P15], F32)
            nc.vector.tensor_mul(p2_sb, pe_sb, pe_sb)
            inv_sb = singles.tile([128, P15], F32)
            nc.vector.reciprocal(inv_sb, p2_sb)
            m3_sb = singles.tile([128, P15], F32)
            nc.vector.scalar_tensor_tensor(
                m3_sb, in0=c1_sb, scalar=1.0, in1=inv_sb, op0=ALU.add, op1=ALU.mult
            )
            m3o_sb = singles.tile([128, P15, P15], F32)
            nc.vector.tensor_mul(
                m3o_sb,
                _with_dims(m3_sb[:], [[1, P15], [0, P15]]),
                _with_dims(m3_sb[:], [[0, P15], [1, P15]]),
            )
            d2fm_sb = singles.tile([128, P15, P15], BF16)
            nc.vector.tensor_mul(d2fm_sb, d2f_sb, m3o_sb)
            tsq_sb = singles.tile([128, P15, R], F32)
            nc.scalar.activation(tsq_sb, t_sb, AF.Square)
            # betas is uniform across R, so -beta folds into the Exp scale
            g_sb = singles.tile([128, P15, R], BF16)
            nc.scalar.activation(g_sb, tsq_sb, AF.Exp, scale=nbs_sb[:, 0:1])
            # sigmoid table load now, well before the tail needs it
            nc.scalar.activation(warm, c_halfpi[:, 0:1], AF.Sigmoid)
            # second copy of g so the two att factors read distinct tiles
            gc_sb = singles.tile([128, P15, R], BF16)
            nc.vector.tensor_copy(gc_sb, g_sb)
            # warmup source: written after `t` so the PE warmup chain starts
            # ~3.5us before the first real transpose and ramps the clock
            warmsrc = singles.tile([128, 128], BF16)
            nc.vector.memset(warmsrc, 0.0)

            # ---------------- att rows, packed [b, 6000] bf16 ----------------
            att_sb = singles.tile([128, FPAD], BF16)
            for a in range(P15):
                cc = P15 - a
                span = cc * R
                off = int(OFFS[a])
                g2_t = g2p.tile([128, P15 * R], BF16, tag="g2")
                nc.vector.tensor_mul(
                    g2_t[:, :span],
                    _with_dims(gc_sb[:, a], [[1, span]]),
                    _bcast(g_sb[:, a], 0, cc),
                )
                nc.vector.tensor_mul(
                    att_sb[:, off:off + span],
                    g2_t[:, :span],
                    _bcast(d2fm_sb[:, a, a:], 1, R),
                )
            nc.vector.memset(att_sb[:, FTOT:], 0.0)

            # PE clock ramp: keep the PE continuously busy from mid-prefix
            # until the first real transpose (idle >3.4us drops it to 1.2GHz)
            ps_warm = ps_warm_pool.tile([128, 128], BF16, tag="warm")
            for _ in range(NWARM):
                nc.tensor.transpose(ps_warm, warmsrc, ident)

            # ---- PE transpose + contraction, groups of GRP chunks ----------
            ps_acc = ps_acc_pool.tile([HID, B], F32)
            for lo in range(0, NCH, GRP):
                hi = min(lo + GRP, NCH)
                pst = ps_t_pool.tile([128, GRP, B], BF16, tag="pst")
                for j in range(lo, hi):
                    nc.tensor.transpose(
                        pst[:, j - lo], att_sb[:, j * 128:(j + 1) * 128], ident
                    )
                attT = attT_pool.tile([128, GRP, B], BF16, tag="attT")
                nc.scalar.copy(attT[:, :hi - lo], pst[:, :hi - lo])
                for j in range(lo, hi):
                    nc.tensor.matmul(
                        ps_acc,
                        lhsT=w1all[:, j],
                        rhs=attT[:, j - lo],
                        start=(j == 0),
                        stop=(j == NCH - 1),
                    )

            # ---------------- + b1/8, transpose to [b, j], ReduceScatter ----
            h1p_sb = singles.tile([HID, B], BF16)
            nc.scalar.activation(h1p_sb, ps_acc, AF.Identity, bias=b18_sb[:, 0:1])
            ps_h1t = ps_t_pool.tile([128, GRP, B], BF16, tag="pst")
            nc.tensor.transpose(ps_h1t[:, 0], h1p_sb, ident)
            h1t_sb = singles.tile([B, HID], BF16)
            nc.vector.tensor_copy(h1t_sb, ps_h1t[:, 0])
            nc.sync.dma_start(out=ar_in, in_=h1t_sb)
            nc.gpsimd.collective_compute(
                "ReduceScatter",
                ALU.add,
                replica_groups=[list(range(n_cores))],
                ins=[ar_in[:].opt()],
                outs=[ar_out[:].opt()],
            )

            # ---------------- local head on 16 molecules ---------------------
            rs_sb = singles.tile([BLOC, HID], BF16)
            nc.sync.dma_start(out=rs_sb, in_=ar_out)
            sg_sb = singles.tile([BLOC, HID], F32)
            nc.scalar.activation(sg_sb, rs_sb, AF.Sigmoid)
            s_sb = singles.tile([BLOC, HID], F32)
            nc.vector.tensor_mul(s_sb, rs_sb, sg_sb)
            prod_sb = singles.tile([BLOC, HID], F32)
            nc.vector.tensor_mul(prod_sb, s_sb, w2r_sb)
            red_sb = singles.tile([BLOC, 1], F32)
            nc.vector.tensor_reduce(
                red_sb, prod_sb, axis=mybir.AxisListType.X, op=ALU.add
            )
            osum_sb = singles.tile([BLOC, 1], F32)
            nc.vector.tensor_scalar(
                osum_sb, red_sb, b2r_sb[:, 0:1], None, op0=ALU.add
            )
            nc.sync.dma_start(out=outd, in_=osum_sb)

    nc.compile()
    return nc


def host_prep(x, W_map, means, betas, W1, b1, W2, b2):
    """Build the 8 per-core input maps (numpy)."""
    import ml_dtypes

    x = np.ascontiguousarray(np.asarray(x, np.float32))
    W_map = np.asarray(W_map, np.float32)
    means = np.asarray(means, np.float32)
    betas = np.asarray(betas, np.float32)
    W1 = np.asarray(W1, np.float32)
    b1 = np.asarray(b1, np.float32).reshape(HID)
    W2 = np.asarray(W2, np.float32).reshape(HID)
    b2 = np.asarray(b2, np.float32).reshape(1)

    # coarse-grained coords per head, computed host-side (trivial FLOPs):
    # xc[h, b, d, k] = sum_n W_map[h,k,n] x[b,n,d]
    xc_h = np.einsum('hkn,bnd->hbdk', W_map, x).astype(np.float32)

    # Fold W1 (H, 36, 36, R, HID) onto the 15 canonical pairs with mirror
    # signs, then onto the 120 upper-triangular pair-pairs (a-major order).
    P36 = K * K
    canon = [(i, j) for i in range(K) for j in range(i + 1, K)]
    a_of = np.array([i * K + j for (i, j) in canon])
    abar = np.array([j * K + i for (i, j) in canon])
    W1r = W1.reshape(H, P36, P36, R, HID)
    W1q = (
        W1r[:, a_of[:, None], a_of[None, :]]
        - W1r[:, a_of[:, None], abar[None, :]]
        - W1r[:, abar[:, None], a_of[None, :]]
        + W1r[:, abar[:, None], abar[None, :]]
    )  # (H, 15, 15, R, HID)
    tri_a, tri_c = np.triu_indices(P15)
    W1t = W1q[:, tri_a, tri_c] + np.where(
        (tri_a != tri_c)[None, :, None, None], W1q[:, tri_c, tri_a], 0.0
    )  # (H, 120, R, HID)
    # x0.25: device gram factors are 2x ref (cutoff computed as cos+1)
    W1flat = (W1t * 0.25).reshape(H, FTOT, HID)
    W1s_dev = np.zeros((H, FPAD, HID), np.float32)
    W1s_dev[:, :FTOT] = W1flat
    W1s_dev = np.ascontiguousarray(W1s_dev.astype(ml_dtypes.bfloat16))

    mrep = np.ascontiguousarray(np.broadcast_to(means, (B, R)), np.float32)
    assert np.all(betas == betas[0]), "kernel folds the uniform beta into Exp"
    nbs = np.full((B, 1), -float(betas[0]), np.float32)
    b18 = np.ascontiguousarray((b1 / 8.0).reshape(HID, 1), np.float32)
    w2r = np.ascontiguousarray(np.broadcast_to(W2, (BLOC, HID)), np.float32)
    b2r = np.full((BLOC, 1), float(b2[0]), np.float32)

    in_maps = []
    for h in range(H):
        in_maps.append(
            dict(
                xcin=np.ascontiguousarray(xc_h[h]),  # (B, 3, K)
                w1s=W1s_dev[h],
                mrep=mrep,
                nbs=nbs,
                b18=b18,
                w2r=w2r,
                b2r=b2r,
            )
        )
    return in_maps


_NC_CACHE = {}


def get_program():
    if "nc" not in _NC_CACHE:
        _NC_CACHE["nc"] = build_program()
    return _NC_CACHE["nc"]


def kernel(x, W_map, means, betas, W1, b1, W2, b2, _debug=False, _trace=False):
    in_maps = host_prep(x, W_map, means, betas, W1, b1, W2, b2)
    nc = get_program()
    res = run_bass_kernel_spmd(nc, in_maps, list(range(H)), trace=_trace)
    out = np.concatenate(
        [np.asarray(res.results[r]["out"], np.float32) for r in range(H)], axis=0
    )
    if _debug or _trace:
        kernel.last_results = res
    return out
